# revision 24
# baseline (speedup 1.0000x reference)
"""Causal multi-head attention on 8 TRN2 NeuronCores.

Problem: B=4, T=2048, C=1024, H=16 heads, D=64. f32 in/out.

Sharding (tensor parallel over heads x batch): core i = (b = i//2, g = i%2)
handles batch b and head-group g (8 heads = 512 channels).  Each core gets
  xt  = x[b].T                      [C, T]   (pre-transposed on host)
  wq/wk/wv = w_qkv column slices    [C, 512]
  wp  = w_proj row slice            [512, C]
and produces a PARTIAL projection output out^T [C, T]; the host sums the two
group partials per batch and transposes back.  No on-device collectives.

Per-core macro-pipeline over t-blocks of 512 (causality makes attention for
query block qb depend only on K/V t-blocks <= qb):
  A(tb): load x^T chunk, cast to bf16, project Q^T,K^T (w-stationary) and
         V (x^T-stationary, natural layout, ones-column appended per head).
  B(qb=tb): per head-pair: S^T[k,q] matmuls (2 heads row-packed via
         tile_position), exp on ScalarE with fused 1/8 scale (valid columns
         only), causal triangle mask via gpsimd affine_select, AV matmuls
         against V_aug -> Y^T with softmax denominator Z in row 64 for free.
         Z rows collect into one [8,512] tile; one exact DVE reciprocal per
         q-block; 1/Z broadcast across partitions via a DRAM round-trip DMA;
         final normalize multiplies write bf16 Y^T.
  C(qb=tb): out^T tile = w_proj-stationary matmul vs Y^T, DVE copy, DMA out.
"""

import numpy as np

B, T, C, H, D = 4, 2048, 1024, 16, 64
G = 2          # head groups (cores per batch)
GC = 512       # channels per group (8 heads * 64)
NCORES = 8
CT = C // 128   # 8 c-tiles
NT = T // 128   # 16 t-tiles of 128
TB = T // 512   # 4 t-blocks of 512
HP = 4          # head-pairs per group

_CACHE = {}


def _build():
    import concourse.bass as bass
    import concourse.tile as tile
    from concourse import bacc, mybir

    f32 = mybir.dt.float32
    bf16 = mybir.dt.bfloat16
    Alu = mybir.AluOpType
    Act = mybir.ActivationFunctionType

    nc = bacc.Bacc("TRN2", target_bir_lowering=False, debug=False,
                   num_devices=NCORES)

    f32r = mybir.dt.float32r
    xt = nc.dram_tensor("xt", [C, T], f32r, kind="ExternalInput").ap()
    wq = nc.dram_tensor("wq", [C, GC], f32r, kind="ExternalInput").ap()
    wk = nc.dram_tensor("wk", [C, GC], f32r, kind="ExternalInput").ap()
    wv = nc.dram_tensor("wv", [C, GC], f32r, kind="ExternalInput").ap()
    wp = nc.dram_tensor("wp", [GC, C], f32, kind="ExternalInput").ap()
    out = nc.dram_tensor("out", [C, T], f32, kind="ExternalOutput").ap()

    xt3 = xt.rearrange("(co p) t -> p co t", p=128)     # [128, 8, T]
    wq3 = wq.rearrange("(co p) n -> p co n", p=128)     # [128, 8, 512]
    wk3 = wk.rearrange("(co p) n -> p co n", p=128)
    wv3 = wv.rearrange("(co p) n -> p co n", p=128)
    wp3 = wp.rearrange("(yo p) n -> p yo n", p=128)     # [128, 4, 1024]
    out3 = out.rearrange("(co p) t -> p co t", p=128)   # [128, 8, T]

    with tile.TileContext(nc) as tc:
        with tc.tile_pool(name="persist", bufs=1) as persist, \
             tc.tile_pool(name="xbp", bufs=2) as xbp, \
             tc.tile_pool(name="ptp", bufs=4) as ptp, \
             tc.tile_pool(name="smal", bufs=4) as smal, \
             tc.tile_pool(name="yub", bufs=6) as yubp, \
             tc.tile_pool(name="ostg", bufs=2) as ostg, \
             tc.tile_pool(name="dramp", bufs=2, space="DRAM") as dramp, \
             tc.tile_pool(name="psA", bufs=2, space="PSUM") as psA, \
             tc.tile_pool(name="st2", bufs=2, space="PSUM") as st2p, \
             tc.tile_pool(name="yap", bufs=2, space="PSUM") as yap:
            # persistent SBUF tensors (per-partition KB in comments)
            wqf = persist.tile([128, CT, GC], f32r)       # 16K
            wkf = persist.tile([128, CT, GC], f32r)       # 16K
            wvf = persist.tile([128, CT, GC], f32r)       # 16K
            wpb = persist.tile([128, 4, C], bf16)         # 8K
            qts = [[persist.tile([128, 512], bf16, name=f"qt{_t}_{_h}")
                    for _h in range(HP)] for _t in range(TB)]     # 16K
            kts = [[persist.tile([128, 512], bf16, name=f"kt{_t}_{_h}")
                    for _h in range(HP)] for _t in range(TB)]     # 16K
            vsbs = [[persist.tile([128, 8, 65], bf16, name=f"vsb{_t}_{_l}")
                     for _l in range(4)] for _t in range(TB)]     # 16.3K
            yts = [[persist.tile([128, 512], bf16, name=f"yt{_t}_{_h}")
                    for _h in range(HP)] for _t in range(TB)]     # 16K

            xtiles = {}

            def dma_cols(dst, src_ap, nchunk=2, engine=None):
                # chunked DMA of the last (column) dim, no cast
                ncols = dst.shape[-1]
                step = ncols // nchunk
                eng = engine or nc.sync
                for i in range(nchunk):
                    csl = slice(i * step, (i + 1) * step)
                    eng.dma_start(out=dst[:, :, csl], in_=src_ap[:, :, csl])

            def alloc_x(tb):
                xtiles[tb] = xbp.tile([128, CT, 512], f32r, tag="xbp",
                                      name=f"xb{tb}")
                return xtiles[tb]

            # ones column of V_aug (gpsimd: off the DVE critical path)
            for _vl in vsbs:
                for _v in _vl:
                    nc.gpsimd.memset(_v[:, :, 64:65], 1.0)

            wp_loaded = [False]

            def a_units(tb):
                units = []

                def qk_unit(wsb, dsts, hp):
                    def f():
                        xb = xtiles[tb]
                        ps = psA.tile([128, 512], f32, tag="psA", name="psA")
                        for c in range(CT):
                            nc.tensor.matmul(
                                out=ps,
                                lhsT=wsb[:, c, hp * 128:hp * 128 + 128],
                                rhs=xb[:, c, :],
                                start=(c == 0), stop=(c == CT - 1))
                        nc.vector.tensor_copy(out=dsts[tb][hp], in_=ps)
                    return f

                def v_unit(tl):
                    def f():
                        xb = xtiles[tb]
                        ps = psA.tile([128, 512], f32, tag="psA", name="psV")
                        for c in range(CT):
                            nc.tensor.matmul(
                                out=ps,
                                lhsT=xb[:, c, tl * 128:tl * 128 + 128],
                                rhs=wvf[:, c, :],
                                start=(c == 0), stop=(c == CT - 1))
                        nc.vector.tensor_copy(
                            out=vsbs[tb][tl][:, :, 0:64],
                            in_=ps.rearrange("p (h d) -> p h d", h=8))
                    return f

                prefix = [qk_unit(wqf, qts, 0), qk_unit(wkf, kts, 0)]
                prefix += [v_unit(tl) for tl in range(4)]
                rest = []
                for hp in range(1, HP):
                    rest.append((hp, qk_unit(wqf, qts, hp)))
                    rest.append((hp, qk_unit(wkf, kts, hp)))
                return prefix, rest

            def b_units(qb):
                units = []
                nk = 4 * qb + 4
                state = {}

                def setup():
                    state["zz"] = [smal.tile([128, 512], f32, tag="zz",
                                             name=f"zz{_i}")
                                   for _i in range(2)]
                    for _z in state["zz"]:
                        nc.gpsimd.memset(_z, 1.0)
                    state["rrs"] = [smal.tile([128, 512], f32, tag="zz",
                                              name=f"rr{_i}")
                                    for _i in range(2)]
                    state["rds"] = [dramp.tile([4, 512], f32, tag="rd",
                                               name=f"rd{_i}")
                                    for _i in range(2)]
                    state["yub"] = [yubp.tile([128, 512], f32, tag="yub",
                                              name=f"yub{_h}")
                                    for _h in range(HP)]

                def hp_start(hp):
                    def f():
                        if hp == 0:
                            setup()
                        state["ya"] = [yap.tile([65, 512], f32, tag="yap",
                                                name=f"ya{_h}")
                                       for _h in range(2)]
                    return f

                def j_unit(hp, j):
                    def f():
                        ya = state["ya"]
                        off = j - 4 * qb
                        v0 = max(0, 128 * off)
                        jt, jl = j // 4, j % 4
                        st2 = st2p.tile([128, 2, 512], f32, tag="st2",
                                        name="st2")
                        for h2 in range(2):
                            p0 = 64 * h2
                            nc.tensor.matmul(
                                out=st2[:, h2, v0:],
                                lhsT=kts[jt][hp][p0:p0 + 64,
                                                 jl * 128:jl * 128 + 128],
                                rhs=qts[qb][hp][p0:p0 + 64, v0:],
                                start=True, stop=True,
                                tile_position=(p0, 0),
                                skip_group_check=True)
                        pt2 = ptp.tile([128, 2, 512], bf16, tag="ptp",
                                       name="pt2")
                        nc.scalar.activation(
                            out=pt2[:, :, v0:], in_=st2[:, :, v0:],
                            func=Act.Exp, scale=0.125)
                        if off >= 0:
                            nc.gpsimd.affine_select(
                                out=pt2[:, :, v0:v0 + 128],
                                in_=pt2[:, :, v0:v0 + 128],
                                pattern=[[0, 2], [1, 128]],
                                compare_op=Alu.is_ge,
                                fill=0.0,
                                base=0,
                                channel_multiplier=-1)
                        for h2 in range(2):
                            nc.tensor.matmul(
                                out=ya[h2][:, v0:],
                                lhsT=vsbs[jt][jl][:, 2 * hp + h2, :],
                                rhs=pt2[:, h2, v0:],
                                start=(j == 0), stop=(j == nk - 1),
                                skip_group_check=True)
                    return f

                def hp_end(hp):
                    def f():
                        ya = state["ya"]
                        zz, rrs, rds = (state["zz"], state["rrs"],
                                        state["rds"])
                        yub = state["yub"]
                        for h2 in range(2):
                            g = 2 * hp + h2
                            row = 32 * (g % 4)
                            nc.scalar.copy(
                                out=zz[g // 4][row:row + 1, :],
                                in_=ya[h2][64:65, :])
                            nc.scalar.copy(
                                out=yub[hp][64 * h2:64 * h2 + 64, :],
                                in_=ya[h2][0:64, :])
                        if hp % 2 == 1:
                            i = hp // 2
                            nc.vector.reciprocal(rrs[i], zz[i])
                            nc.sync.dma_start(
                                out=rds[i],
                                in_=rrs[i].rearrange("(a b) n -> a b n",
                                                     b=32)[:, 0, :])
                    return f

                def norm_unit(hp):
                    def f():
                        rds, yub = state["rds"], state["yub"]
                        i, g0, g1 = hp // 2, 2 * hp, 2 * hp + 1
                        rb = smal.tile([128, 512], f32, tag="rb", name="rb")
                        nc.sync.dma_start(
                            out=rb[0:64],
                            in_=rds[i][g0 % 4:g0 % 4 + 1]
                            .to_broadcast([64, 512]))
                        nc.sync.dma_start(
                            out=rb[64:128],
                            in_=rds[i][g1 % 4:g1 % 4 + 1]
                            .to_broadcast([64, 512]))
                        nc.vector.tensor_mul(
                            out=yts[qb][hp],
                            in0=yub[hp],
                            in1=rb)
                    return f

                for hp in range(HP):
                    units.append(hp_start(hp))
                    for j in range(nk):
                        units.append(j_unit(hp, j))
                    units.append(hp_end(hp))
                    if hp == 1:
                        units.append(norm_unit(0))
                        units.append(norm_unit(1))
                units.append(norm_unit(2))
                units.append(norm_unit(3))
                return units

            def c_units(qb):
                units = []

                def co_unit(co):
                    def f():
                        ps = psA.tile([128, 512], f32, tag="psA",
                                      name="psC")
                        for yti in range(4):
                            nc.tensor.matmul(
                                out=ps,
                                lhsT=wpb[:, yti, co * 128:co * 128 + 128],
                                rhs=yts[qb][yti],
                                start=(yti == 0), stop=(yti == 3))
                        ob = ostg.tile([128, 512], f32, tag="ostg",
                                       name="ob")
                        if qb == TB - 1:
                            nc.scalar.copy(out=ob, in_=ps)
                        else:
                            nc.vector.tensor_copy(out=ob, in_=ps)
                        nc.sync.dma_start(
                            out=out3[:, co, qb * 512:qb * 512 + 512],
                            in_=ob)
                    return f

                for co in range(CT):
                    units.append(co_unit(co))
                return units

            def load_units(tb):
                def f():
                    xb = alloc_x(tb)
                    dma_cols(xb, xt3[:, :, tb * 512:tb * 512 + 512],
                             nchunk=4)
                return [f]

            def wp_unit():
                def f():
                    s = xbp.tile([128, 4, C], f32, tag="xbp", name="s_wp")
                    nc.sync.dma_start(out=s, in_=wp3)
                    nc.vector.tensor_copy(out=wpb, in_=s)
                return [f]

            def interleave(primary, deadlined, free):
                # primary: list of thunks; deadlined: list of
                # (primary_index_deadline, thunk) emitted BEFORE that index
                # (emission order defines dependencies!); free: thunks
                # sprinkled proportionally.
                di = fi = 0
                for i, u in enumerate(primary):
                    while di < len(deadlined) and deadlined[di][0] <= i:
                        deadlined[di][1]()
                        di += 1
                    u()
                    want = (i + 1) * len(free) // len(primary)
                    while fi < min(want, len(free)):
                        free[fi]()
                        fi += 1
                while di < len(deadlined):
                    deadlined[di][1]()
                    di += 1
                while fi < len(free):
                    free[fi]()
                    fi += 1

            # flat pipeline: B(tb)+C(tb) interleaved with the rest of
            # A(tb) (Q/K for hp>=1, deadline-ordered before the B units
            # that read them) and the prefix of A(tb+1)
            prefixes = {}
            rests = {}
            prefixes[0], rests[0] = a_units(0)
            # emission order follows need order: Q-hp0 needs wq cols 0:256
            # + all of x0; K-hp0 needs wk cols 0:256; V needs all of wv.
            xb0 = alloc_x(0)
            nc.sync.dma_start(out=wqf[:, :, 0:256], in_=wq3[:, :, 0:256])
            dma_cols(xb0, xt3[:, :, 0:512], nchunk=4)
            p0 = prefixes[0]
            p0[0]()                                   # Q-hp0
            nc.sync.dma_start(out=wkf[:, :, 0:256], in_=wk3[:, :, 0:256])
            p0[1]()                                   # K-hp0
            dma_cols(wvf, wv3)
            for u in p0[2:]:                          # V units
                u()
            nc.sync.dma_start(out=wqf[:, :, 256:512], in_=wq3[:, :, 256:512])
            nc.sync.dma_start(out=wkf[:, :, 256:512], in_=wk3[:, :, 256:512])
            for tb in range(TB):
                nk = 4 * tb + 4
                deadlined = [(max(0, hp * (nk + 2) - 2), u)
                             for hp, u in rests[tb]]
                free = []
                if tb == 0:
                    free += wp_unit()
                if tb > 0:
                    free += c_units(tb - 1)   # C fills the next stage
                if tb + 1 < TB:
                    free += load_units(tb + 1)
                    prefixes[tb + 1], rests[tb + 1] = a_units(tb + 1)
                    free += prefixes[tb + 1]
                interleave(b_units(tb), deadlined, free)
            for u in c_units(TB - 1):
                u()

    nc.compile()
    return nc


def _get_nc():
    if "nc" not in _CACHE:
        _CACHE["nc"] = _build()
    return _CACHE["nc"]


def _make_in_maps(x, w_qkv, w_proj):
    x = np.asarray(x, dtype=np.float32)
    w_qkv = np.asarray(w_qkv, dtype=np.float32)
    w_proj = np.asarray(w_proj, dtype=np.float32)
    in_maps = []
    for i in range(NCORES):
        b, g = divmod(i, G)
        cs = slice(g * GC, (g + 1) * GC)
        in_maps.append({
            "xt": np.ascontiguousarray(x[b].T),
            "wq": np.ascontiguousarray(w_qkv[:, cs]),
            "wk": np.ascontiguousarray(w_qkv[:, C + g * GC:C + (g + 1) * GC]),
            "wv": np.ascontiguousarray(
                w_qkv[:, 2 * C + g * GC:2 * C + (g + 1) * GC]),
            "wp": np.ascontiguousarray(w_proj[cs, :]),
        })
    return in_maps


def _run(x, w_qkv, w_proj, trace=False):
    from concourse.bass_utils import run_bass_kernel_spmd
    nc = _get_nc()
    res = run_bass_kernel_spmd(nc, _make_in_maps(x, w_qkv, w_proj),
                               core_ids=list(range(NCORES)), trace=trace)
    outs = [np.asarray(r["out"], dtype=np.float32) for r in res.results]
    full = np.empty((B, T, C), dtype=np.float32)
    for b in range(B):
        full[b] = (outs[2 * b] + outs[2 * b + 1]).T
    return full, res


def kernel(x, w_qkv, w_proj):
    full, _ = _run(x, w_qkv, w_proj, trace=False)
    return full


def _install_trace_shims():
    """The agent image lacks antenv.axon_hooks; recreate the NTFF hook the
    axon boot would have registered, and skip the artifact upload (no
    network egress here)."""
    import sys
    import types

    import antenv
    from concourse import bass_utils

    bass_utils.upload_artifacts = lambda tmpdir: tmpdir
    if "antenv.axon_hooks" not in sys.modules:
        import os as _os

        from trn_agent_boot import trn_boot
        hook = trn_boot._ntff_profile_via_ctypes(
            _os.environ.get("PJRT_LIBRARY_PATH", "/opt/axon/libaxon_pjrt.so"))
        mod = types.ModuleType("antenv.axon_hooks")
        mod.get_axon_ntff_profile_hook = lambda: hook
        mod.set_axon_ntff_profile_hook = lambda h: None
        sys.modules["antenv.axon_hooks"] = mod
        antenv.axon_hooks = mod


def bench(x, w_qkv, w_proj):
    """Returns (output, exec_time_ns)."""
    _install_trace_shims()
    full, res = _run(x, w_qkv, w_proj, trace=True)
    return full, res.exec_time_ns
